# revision 10
# baseline (speedup 1.0000x reference)
"""Trainium2 Bass kernel for nn_DigitConvolutionalModel.

Model: x(B,784) -> reshape 28x28 -> 3x3 valid cross-correlation (kernel is an
input) -> flatten 676 -> Linear(676,128)+ReLU -> Linear(128,10).

Strategy:
  * Fold the 3x3 conv into the first linear layer on the host: the conv is a
    linear map, so h = relu(x @ W1eff.T + b1) with W1eff (128, 784) built by
    scattering conv_w-weighted copies of w1 onto the 28x28 grid. The device
    kernel is then a plain 2-layer MLP over 784 features.
  * Pure data parallelism: batch 65536 split as 8192 rows per NeuronCore,
    weights replicated.
  * Activations are shipped feature-major and fp16 (the PE runs fp16 at full
    rate and the per-core HBM ceiling is the bottleneck, so halving bytes
    halves the kernel time; measured end-to-end error ~5e-4 of scale).
    The kernel computes logits^T = w2 @ relu(W1eff @ x^T + b1) + b2 and the
    host transposes the gathered (10, B) result back.
  * x is shipped pre-packed chunk-minor per block: partition p holds its 7
    contraction chunks back-to-back, so a block load is 112 fully-contiguous
    per-partition runs of 7*xb elements (28 KB at xb=2048) - large
    descriptors keep the SDMA engines at high efficiency.
  * Engine roles are kept disjoint to avoid head-of-line blocking: sync ring
    does all x loads, ACT does weight loads + output bias, DVE does
    relu+bias1, gpsimd (SWDGE) streams the per-block outputs out, PE does
    matmuls only.
  * The PE clock is HAM-gated (cold 1.2 GHz until ~3.4us of sustained
    activity): a burst of warmup matmuls on scratch data during the DMA
    ramp, plus a few filler matmuls per block gap, keep the array at
    2.4 GHz for the real work.
"""

from contextlib import ExitStack

import numpy as np

B = 65536
H = W = 28
K = 3
CH = CW = 26
FEAT = H * W          # 784
HID = 128
OUT = 10
NCORES = 8
BC = B // NCORES      # 8192 rows per core

KC = 112              # contraction-chunk partition size
KCH = 7               # chunks: 7 * 112 = 784
NT = 512              # max batch rows per compute tile (one PSUM bank fp32)
XB = 1024             # generic block size for non-8192 (smoke) builds

N_WARM = 20           # HAM warmup matmuls before the first block lands
NT_WARM = 256         # warmup matmul free dim: fine-grained so the queue
                      # drains quickly once real work is ready

VARIANT = "f16"

_NC_CACHE = {}


def _blocks(bc):
    # many small blocks, round-robined across both HWDGE rings: keeps both
    # DMA queues streaming and lets compute trickle in per block instead of
    # bursting (which would HAM-throttle the PE between bursts); small final
    # blocks shorten the post-DMA compute tail
    if bc == 8192:
        blocks = [512] * 15 + [256, 128, 128]
    else:
        blocks = [min(XB, bc - o) for o in range(0, bc, XB)]
    assert sum(blocks) == bc
    return blocks


def _tiles(xb):
    out, t0 = [], 0
    while t0 < xb:
        nt = min(NT, xb - t0)
        out.append((t0, nt))
        t0 += nt
    return out


def _dtypes(variant):
    import concourse.mybir as mybir

    f32 = mybir.dt.float32
    if variant == "f32":
        return f32, f32
    if variant == "bf16":
        return mybir.dt.bfloat16, mybir.dt.bfloat16
    if variant == "f16":
        return mybir.dt.float16, mybir.dt.float16
    raise ValueError(variant)


def _build_nc(bc, variant):
    from concourse import bacc
    import concourse.mybir as mybir
    import concourse.tile as tile

    f32 = mybir.dt.float32
    wdt, xdt = _dtypes(variant)
    blocks = _blocks(bc)
    nblk = len(blocks)

    nc = bacc.Bacc(
        "TRN2",
        target_bir_lowering=False,
        debug=False,
        enable_asserts=False,
        num_devices=NCORES,
        enable_partition_id=False,
    )
    # [112, 7*bc] chunk-minor packed per block: within block (off, xb),
    # partition p holds chunks 0..6 contiguously (xb elements each), so the
    # block load is one DMA of 112 contiguous per-partition runs
    xT = nc.dram_tensor("xT", [KC, KCH * bc], xdt, kind="ExternalInput").ap()
    w1t = nc.dram_tensor("w1t", [KC, KCH, HID], wdt, kind="ExternalInput").ap()
    b1 = nc.dram_tensor("b1", [HID, 1], f32, kind="ExternalInput").ap()
    w2t = nc.dram_tensor("w2t", [HID, OUT], wdt, kind="ExternalInput").ap()
    b2 = nc.dram_tensor("b2", [OUT, 1], f32, kind="ExternalInput").ap()
    outT = nc.dram_tensor("outT", [OUT, bc], f32, kind="ExternalOutput").ap()

    relu = mybir.ActivationFunctionType.Relu

    with ExitStack() as ctx:
        tc = ctx.enter_context(tile.TileContext(nc))
        wpool = ctx.enter_context(tc.tile_pool(name="w", bufs=1))
        # deep x buffering: the DMA streams keep running through transient
        # compute lag (cold-PE phases) instead of stalling on tile reuse
        xpool = ctx.enter_context(tc.tile_pool(name="x", bufs=12))
        hpool = ctx.enter_context(tc.tile_pool(name="h", bufs=3))
        p1pool = ctx.enter_context(tc.tile_pool(name="p1", bufs=5, space="PSUM"))
        p2pool = ctx.enter_context(tc.tile_pool(name="p2", bufs=2, space="PSUM"))
        pwpool = ctx.enter_context(tc.tile_pool(name="pw", bufs=1, space="PSUM"))

        # weights ride the scalar HWDGE ring ahead of its first x block (the
        # SWDGE path was measured ~2us/load serialized on Q7 — too slow for
        # w1s, which gates the first real matmul)
        w1s = wpool.tile([KC, KCH, HID], wdt)
        nc.scalar.dma_start(w1s[:], w1t[:])
        b1s = wpool.tile([HID, 1], f32)
        nc.scalar.dma_start(b1s[:], b1[:])
        w2s = wpool.tile([HID, OUT], wdt)
        nc.scalar.dma_start(w2s[:], w2t[:])
        b2s = wpool.tile([OUT, 1], f32)
        nc.scalar.dma_start(b2s[:], b2[:])

        # whole per-core output stays resident in SBUF (32 KB/partition on 10
        # partitions); one final store instead of per-block stores
        os_ = wpool.tile([OUT, bc], f32)

        # HAM warmup: scratch matmuls keep the PE busy from the preamble until
        # the first x block lands, so real matmuls run at 2.4 GHz
        ws = wpool.tile([KC, NT_WARM], xdt)
        nc.gpsimd.memset(ws[:], 0.0)
        pw = pwpool.tile([HID, NT_WARM], f32)
        for _ in range(N_WARM):
            nc.tensor.matmul(pw[:], ws[:, :HID], ws[:], start=True, stop=True)

        rings = [nc.sync, nc.scalar, nc.gpsimd]
        off = 0
        for blk, xb in enumerate(blocks):
            tts = _tiles(xb)
            if blk >= nblk - 2:
                # tail blocks ride the HWDGE rings (no SWDGE descriptor-gen
                # latency on the critical path)
                ring = nc.sync if blk == nblk - 1 else nc.scalar
            else:
                ring = rings[blk % 3]
            xs = xpool.tile([KC, KCH * xb], xdt, tag="xs", name=f"xs_{blk}")
            ring.dma_start(xs[:], xT[:, KCH * off : KCH * (off + xb)])
            p1s = [
                p1pool.tile([HID, nt], f32, tag="p1", name=f"p1_{blk}_{i}")
                for i, (t0, nt) in enumerate(tts)
            ]
            for c in range(KCH):
                for i, (t0, nt) in enumerate(tts):
                    nc.tensor.matmul(
                        p1s[i][:],
                        w1s[:, c, :],
                        xs[:, c * xb + t0 : c * xb + t0 + nt],
                        start=(c == 0),
                        stop=(c == KCH - 1),
                    )
            for i, (t0, nt) in enumerate(tts):
                # epilogue on DVE: relu+bias1, then bias2 after the layer-2
                # matmul, accumulated into the resident output tile
                hs = hpool.tile([HID, nt], xdt, tag="hs", name=f"hs_{blk}_{i}")
                nc.vector.tensor_scalar(
                    hs[:], p1s[i][:], b1s[:], 0.0,
                    mybir.AluOpType.add, mybir.AluOpType.max,
                )
                p2 = p2pool.tile([OUT, nt], f32, tag="p2", name=f"p2_{blk}_{i}")
                nc.tensor.matmul(p2[:], w2s[:], hs[:], start=True, stop=True)
                nc.vector.tensor_scalar_add(
                    os_[:, off + t0 : off + t0 + nt], p2[:], b2s[:]
                )
            off += xb
        # single tail store on the sync HWDGE ring (idle by now)
        nc.sync.dma_start(outT[:], os_[:])

    nc.compile()
    return nc


def get_nc(bc=BC, variant=VARIANT):
    key = (bc, variant)
    if key not in _NC_CACHE:
        _NC_CACHE[key] = _build_nc(bc, variant)
    return _NC_CACHE[key]


def _np_wdt(variant):
    if variant == "bf16":
        import ml_dtypes

        return ml_dtypes.bfloat16
    if variant == "f16":
        return np.float16
    return np.float32


def _pack_xT(shardT, blocks):
    """[784, bc] feature-major shard -> [112, 7*bc] per-block chunk-minor."""
    bc = shardT.shape[1]
    a = shardT.reshape(KCH, KC, bc)  # [c, p, :] holds feature c*112+p
    parts = []
    off = 0
    for xb in blocks:
        parts.append(
            a[:, :, off : off + xb].transpose(1, 0, 2).reshape(KC, KCH * xb)
        )
        off += xb
    return np.concatenate(parts, axis=1)


def _host_prep(x, conv_w, w1, b1, w2, b2, variant):
    """Fold conv into layer-1 weights and lay out per-core device inputs."""
    x = np.asarray(x, dtype=np.float32)
    conv_w = np.asarray(conv_w, dtype=np.float32)
    w1 = np.asarray(w1, dtype=np.float32)
    b1 = np.asarray(b1, dtype=np.float32)
    w2 = np.asarray(w2, dtype=np.float32)
    b2 = np.asarray(b2, dtype=np.float32)

    w1_img = w1.reshape(HID, CH, CW)
    w1eff = np.zeros((HID, H, W), dtype=np.float32)
    for di in range(K):
        for dj in range(K):
            w1eff[:, di : di + CH, dj : dj + CW] += conv_w[di, dj] * w1_img
    w1eff = w1eff.reshape(HID, FEAT)

    wnp = _np_wdt(variant)
    # [784,128] -> [7,112,128] -> [112,7,128] so chunk c partition p holds
    # feature c*112+p
    w1t_host = np.ascontiguousarray(
        w1eff.T.reshape(KCH, KC, HID).transpose(1, 0, 2)
    ).astype(wnp)
    b1_host = np.ascontiguousarray(b1.reshape(HID, 1))
    w2t_host = np.ascontiguousarray(w2.T).astype(wnp)
    b2_host = np.ascontiguousarray(b2.reshape(OUT, 1))

    blocks = _blocks(BC)
    xh = x.astype(wnp)  # cast once, then rearrange in the narrow dtype
    in_maps = []
    for c in range(NCORES):
        shardT = np.ascontiguousarray(xh[c * BC : (c + 1) * BC].T)  # [784, BC]
        in_maps.append(
            {
                "xT": _pack_xT(shardT, blocks),
                "w1t": w1t_host,
                "b1": b1_host,
                "w2t": w2t_host,
                "b2": b2_host,
            }
        )
    return in_maps


def run(x, conv_w, w1, b1, w2, b2, trace=False, variant=VARIANT):
    from concourse.bass_utils import run_bass_kernel_spmd

    in_maps = _host_prep(x, conv_w, w1, b1, w2, b2, variant)
    nc = get_nc(BC, variant)
    res = run_bass_kernel_spmd(nc, in_maps, list(range(NCORES)), trace=trace)
    outT = np.concatenate([r["outT"] for r in res.results], axis=1)  # [10, B]
    return np.ascontiguousarray(outT.T), res


def kernel(x, conv_w, w1, b1, w2, b2):
    out, _ = run(x, conv_w, w1, b1, w2, b2)
    return out
